# revision 18
# baseline (speedup 1.0000x reference)
"""Trainium2 Bass kernel for autoregressive GRU sampling.

Problem: B=16384 samples, 1024 sequential sites, hidden=64, PyTorch GRU-cell
math with gates [r,z,n], Bernoulli sampling via pre-drawn uniforms.

Strategy (v4):
  - Pure data parallel over 8 cores (2048 samples/core); 4 phase-shifted
    streams of 512 columns per core.
  - All matmuls run as bf16 hi/lo pair-split passes (1 cycle/col each,
    vs 8 cycles/col for fp32): X*W ~= Xhi*Whi + Xlo*Whi + Xhi*Wlo, with
    W = Whi + Wlo split exactly on the host and X = h' split on device
    (hi = bf16(h'), lo = bf16(h' - hi)). Error ~2^-18, checked empirically
    against the 2e-2 relative-error budget on the fixed problem seed.
  - Moving tiles per stream (bf16): M1 = [hhi(0-63), s(64), one(65),
    uhat_hi(66)], M2 = [hlo(0-63), 0, 0, uhat_mid(66), uhat_lo(67)].
    Gate passes read rows 0-65 (M2's zero rows avoid double-counting the
    s/one contributions); the head pass reads the uhat rows too, with
    stationary -1 entries so psum = head.h' - uhat (uhat as an exact
    bf16 triple). One ACT Sign then yields s' = sign(logit - uhat).
  - The s in {-1,+1} encoding makes bit*w_ih = s*(w/2) + w/2 ride the
    stationary rows exactly (w/2 is lossless); biases ride row 65.
  - fp32 state h' persists only for the GRU combine (d = h - n).
"""

import numpy as np
from contextlib import ExitStack

HIDDEN = 64
N_SITES = 1024
BATCH = 16384
N_CORES = 8
B_LOCAL = BATCH // N_CORES  # 2048
G = 4                       # streams per core
W = B_LOCAL // G            # 512 columns per stream
NSTAGE = 13
PHASE = (0, 1, 5, 6)        # per-stream stage offsets

_BUILD_CACHE = {}


def _build(n_sites: int, compile: bool = True):
    import concourse.bacc as bacc
    import concourse.tile as tile
    from concourse import mybir

    f32 = mybir.dt.float32
    bf16 = mybir.dt.bfloat16
    AF = mybir.ActivationFunctionType
    OP = mybir.AluOpType

    nc = bacc.Bacc()
    u1_d = nc.dram_tensor("u1", [n_sites, B_LOCAL], bf16, kind="ExternalInput")
    u2_d = nc.dram_tensor("u2", [n_sites, 2, B_LOCAL], bf16, kind="ExternalInput")
    wrzh_d = nc.dram_tensor("wrzh", [66, 128], bf16, kind="ExternalInput")
    wrzl_d = nc.dram_tensor("wrzl", [66, 128], bf16, kind="ExternalInput")
    wnh_d = nc.dram_tensor("wnh", [66, 128], bf16, kind="ExternalInput")
    wnl_d = nc.dram_tensor("wnl", [66, 128], bf16, kind="ExternalInput")
    h1h_d = nc.dram_tensor("h1h", [67, 1], bf16, kind="ExternalInput")
    h1l_d = nc.dram_tensor("h1l", [67, 1], bf16, kind="ExternalInput")
    h2_d = nc.dram_tensor("h2", [68, 1], bf16, kind="ExternalInput")
    bits_d = nc.dram_tensor("bits", [n_sites, B_LOCAL], bf16, kind="ExternalOutput")

    with ExitStack() as ctx:
        tc = ctx.enter_context(tile.TileContext(nc))
        const = ctx.enter_context(tc.tile_pool(name="const", bufs=1))
        work = ctx.enter_context(tc.tile_pool(name="work", bufs=2))
        ps_rz = ctx.enter_context(tc.tile_pool(name="psrz", bufs=1, space="PSUM"))
        ps_n = ctx.enter_context(tc.tile_pool(name="psn", bufs=1, space="PSUM"))
        ps_hd = ctx.enter_context(tc.tile_pool(name="pshd", bufs=1, space="PSUM"))

        # Weights bounce through DVE copies to keep consumer wait-sets small.
        names = [
            ("wrzh", [66, 128]), ("wrzl", [66, 128]),
            ("wnh", [66, 128]), ("wnl", [66, 128]),
            ("h1h", [67, 1]), ("h1l", [67, 1]), ("h2", [68, 1]),
        ]
        drams = dict(wrzh=wrzh_d, wrzl=wrzl_d, wnh=wnh_d, wnl=wnl_d,
                     h1h=h1h_d, h1l=h1l_d, h2=h2_d)
        wt = {}
        for nm, shp in names:
            raw = const.tile(shp, bf16, name=nm + "_raw")
            dst = const.tile(shp, bf16, name=nm)
            nc.sync.dma_start(raw[:], drams[nm][:])
            nc.vector.tensor_copy(dst[:], raw[:])
            wt[nm] = dst

        # fp32 h' state (pure h rows), ping-pong per stream.
        hstate = [
            [const.tile([64, W], f32, name=f"hs{g}_{p}") for p in range(2)]
            for g in range(G)
        ]
        for g in range(G):
            for p in range(2):
                nc.vector.memzero(hstate[g][p][:])

        # M1/M2 as explicit stable double-buffers (slot t -> buffer t%2):
        # stable tensors let Tile see every cross-site WAR/RAW conflict
        # (a rotating pool ring would be blind to readers emitted after the
        # next allocation, which is a real hardware race).
        m1b = [
            [const.tile([67, W], bf16, name=f"m1_{g}_{p}") for p in range(2)]
            for g in range(G)
        ]
        m2b = [
            [const.tile([68, W], bf16, name=f"m2_{g}_{p}") for p in range(2)]
            for g in range(G)
        ]

        def m1s(g, t):
            return m1b[g][t % 2]

        def m2s(g, t):
            return m2b[g][t % 2]

        def uhat_dma(g, t):
            # u2 first so the head M1-pass's SP wait (on u1) also covers u2.
            nc.sync.dma_start(
                m2s(g, t)[66:68, :], u2_d[t : t + 1, :, g * W : (g + 1) * W]
            )
            nc.sync.dma_start(
                m1s(g, t)[66:67, :], u1_d[t : t + 1, g * W : (g + 1) * W]
            )

        # Prologue. Buffer 1 doubles as the initial state (slot -1): h=0,
        # s=-1 (bit 0), one=1. Buffer 0's s-row is overwritten by sign(0)
        # before gates(1) read it, so 1.0 there is fine (32-aligned memsets).
        for g in range(G):
            m1a, m2a = m1b[g][1], m2b[g][1]
            nc.vector.memzero(m1a[0:64, :])
            nc.vector.memset(m1a[64:66, :], 1.0)
            nc.vector.memset(m1a[64:65, :], -1.0)
            nc.vector.memzero(m2a[0:66, :])
            nc.vector.memset(m1b[g][0][64:66, :], 1.0)
            nc.vector.memzero(m2b[g][0][64:66, :])
            uhat_dma(g, 0)
            uhat_dma(g, 1)

        # Stream g's logit-uhat row: tile hd[g//2], row 32*(g%2).
        hd = [ps_hd.tile([64, W], f32, name=f"hd{i}") for i in range(2)]
        tiles = [{} for _ in range(G)]

        def emit(g, t, stage):
            tl = tiles[g]
            if stage == 0:
                m1p, m2p = m1s(g, t - 1), m2s(g, t - 1)
                # M2 pass first: its DVE RAW (hlo) merges with the psum-WAR
                # DVE wait, keeping every matmul within 2 sync waits.
                pn = ps_n.tile([128, W], f32, tag=f"n{g}", name=f"pn{g}")
                tl["pn"] = pn
                nc.tensor.matmul(pn[:], wt["wnh"][:], m2p[0:66, :], start=True, stop=False)
                nc.tensor.matmul(pn[:], wt["wnh"][:], m1p[0:66, :], start=False, stop=False)
                nc.tensor.matmul(pn[:], wt["wnl"][:], m1p[0:66, :], start=False, stop=True)
                prz = ps_rz.tile([128, W], f32, tag=f"rz{g % 2}", name=f"prz{g}")
                tl["prz"] = prz
                nc.tensor.matmul(prz[:], wt["wrzh"][:], m2p[0:66, :], start=True, stop=False)
                nc.tensor.matmul(prz[:], wt["wrzh"][:], m1p[0:66, :], start=False, stop=False)
                nc.tensor.matmul(prz[:], wt["wrzl"][:], m1p[0:66, :], start=False, stop=True)
            elif stage == 1:
                tl["sg"] = work.tile([128, W], f32, tag=f"sg{g}", name=f"sg{g}")
                nc.scalar.activation(tl["sg"][:], tl["prz"][:], AF.Sigmoid)
            elif stage == 2:
                tl["rg"] = work.tile([64, W], f32, tag=f"rg{g}", name=f"rg{g}")
                nc.vector.scalar_tensor_tensor(
                    tl["rg"][:], tl["pn"][0:64, :], 0.0, tl["sg"][64:128, :],
                    OP.add, OP.mult,
                )
            elif stage == 3:
                tl["np"] = work.tile([64, W], f32, tag=f"np{g}", name=f"np{g}")
                nc.vector.tensor_add(tl["np"][:], tl["pn"][64:128, :], tl["rg"][:])
            elif stage == 4:
                tl["nt"] = work.tile([64, W], f32, tag=f"nt{g}", name=f"nt{g}")
                nc.scalar.activation(tl["nt"][:], tl["np"][:], AF.Tanh)
            elif stage == 5:
                tl["d"] = work.tile([64, W], f32, tag=f"d{g}", name=f"d{g}")
                nc.vector.tensor_sub(tl["d"][:], hstate[g][t % 2][:], tl["nt"][:])
            elif stage == 6:
                tl["zd"] = work.tile([64, W], f32, tag=f"zd{g}", name=f"zd{g}")
                nc.gpsimd.tensor_mul(tl["zd"][:], tl["sg"][0:64, :], tl["d"][:])
            elif stage == 7:
                nc.vector.tensor_add(
                    hstate[g][(t + 1) % 2][:], tl["nt"][:], tl["zd"][:]
                )
            elif stage == 8:
                # hhi = bf16(h') into this site's M1 (allocated at stage 12
                # of site t-2, or in the prologue).
                nc.scalar.activation(
                    m1s(g, t)[0:64, :], hstate[g][(t + 1) % 2][:], AF.Copy
                )
            elif stage == 9:
                nc.vector.tensor_sub(
                    m2s(g, t)[0:64, :], hstate[g][(t + 1) % 2][:], m1s(g, t)[0:64, :]
                )
            elif stage == 10:
                row = 32 * (g % 2)
                o = hd[g // 2][row : row + 1, :]
                nc.tensor.matmul(o, wt["h1h"][:], m1s(g, t)[0:67, :], start=True, stop=False)
                nc.tensor.matmul(o, wt["h2"][:], m2s(g, t)[0:68, :], start=False, stop=False)
                nc.tensor.matmul(o, wt["h1l"][:], m1s(g, t)[0:67, :], start=False, stop=True)
            elif stage == 11:
                row = 32 * (g % 2)
                nc.scalar.activation(
                    m1s(g, t)[64:65, :], hd[g // 2][row : row + 1, :], AF.Sign
                )
            elif stage == 12:
                nc.sync.dma_start(
                    bits_d[t : t + 1, g * W : (g + 1) * W], m1s(g, t)[64:65, :]
                )
                if t + 2 < n_sites:
                    uhat_dma(g, t + 2)

        total = n_sites * NSTAGE
        for tick in range(total + PHASE[-1] + 1):
            for g in range(G):
                k = tick - PHASE[g]
                if 0 <= k < total:
                    t, stage = divmod(k, NSTAGE)
                    emit(g, t, stage)

    if compile:
        nc.compile()
    return nc


def _bf16_split(a32):
    import ml_dtypes

    hi = a32.astype(ml_dtypes.bfloat16)
    lo = (a32 - hi.astype(np.float32)).astype(ml_dtypes.bfloat16)
    return hi, lo


def _pack_inputs(u, w_ih, w_hh, b_ih, b_hh, head_w, head_b):
    import ml_dtypes

    H = HIDDEN
    w_ih = np.asarray(w_ih, np.float64)
    w_hh = np.asarray(w_hh, np.float32)
    b_ih = np.asarray(b_ih, np.float64)
    b_hh = np.asarray(b_hh, np.float64)
    head_w = np.asarray(head_w, np.float32)
    head_b = np.asarray(head_b, np.float64)

    r_sl = slice(0, H)
    z_sl = slice(H, 2 * H)
    n_sl = slice(2 * H, 3 * H)

    # Row 64 multiplies s=2*bit-1, row 65 multiplies const 1:
    #   bit*w = s*(w/2) + w/2, so row64 = w/2 and +w/2 joins the bias.
    wrz = np.zeros((66, 128), np.float32)
    wrz[0:64, 0:64] = w_hh[z_sl].T
    wrz[64, 0:64] = (w_ih[z_sl, 0] / 2).astype(np.float32)
    wrz[65, 0:64] = (b_ih[z_sl] + b_hh[z_sl] + w_ih[z_sl, 0] / 2).astype(np.float32)
    wrz[0:64, 64:128] = w_hh[r_sl].T
    wrz[64, 64:128] = (w_ih[r_sl, 0] / 2).astype(np.float32)
    wrz[65, 64:128] = (b_ih[r_sl] + b_hh[r_sl] + w_ih[r_sl, 0] / 2).astype(np.float32)

    wn = np.zeros((66, 128), np.float32)
    wn[0:64, 0:64] = w_hh[n_sl].T
    wn[65, 0:64] = b_hh[n_sl].astype(np.float32)
    wn[64, 64:128] = (w_ih[n_sl, 0] / 2).astype(np.float32)
    wn[65, 64:128] = (b_ih[n_sl] + w_ih[n_sl, 0] / 2).astype(np.float32)

    wrzh, wrzl = _bf16_split(wrz)
    wnh, wnl = _bf16_split(wn)

    headh, headl = _bf16_split(head_w[0, :].astype(np.float32))
    h1h = np.zeros((67, 1), ml_dtypes.bfloat16)
    h1h[0:64, 0] = headh
    h1h[66, 0] = ml_dtypes.bfloat16(-1.0)
    h1l = np.zeros((67, 1), ml_dtypes.bfloat16)
    h1l[0:64, 0] = headl
    h2 = np.zeros((68, 1), ml_dtypes.bfloat16)
    h2[0:64, 0] = headh
    h2[66, 0] = ml_dtypes.bfloat16(-1.0)
    h2[67, 0] = ml_dtypes.bfloat16(-1.0)

    # uhat = logit(u) - head_b as an exact bf16 triple.
    u64 = np.asarray(u, np.float64)
    L = (np.log(u64) - np.log1p(-u64) - float(head_b[0])).astype(np.float32)
    uhi = L.astype(ml_dtypes.bfloat16)
    rem = L - uhi.astype(np.float32)
    umid = rem.astype(ml_dtypes.bfloat16)
    ulo = (rem - umid.astype(np.float32)).astype(ml_dtypes.bfloat16)

    u1s, u2s = [], []
    for c in range(N_CORES):
        sl = slice(c * B_LOCAL, (c + 1) * B_LOCAL)
        u1s.append(np.ascontiguousarray(uhi[sl].T))
        u2s.append(
            np.ascontiguousarray(
                np.stack([umid[sl].T, ulo[sl].T], axis=1)  # [n_sites, 2, BL]
            )
        )
    return wrzh, wrzl, wnh, wnl, h1h, h1l, h2, u1s, u2s


def _make_in_maps(packed):
    wrzh, wrzl, wnh, wnl, h1h, h1l, h2, u1s, u2s = packed
    return [
        {
            "u1": u1s[c], "u2": u2s[c], "wrzh": wrzh, "wrzl": wrzl,
            "wnh": wnh, "wnl": wnl, "h1h": h1h, "h1l": h1l, "h2": h2,
        }
        for c in range(N_CORES)
    ]


def kernel(u, w_ih, w_hh, b_ih, b_hh, head_w, head_b):
    from concourse.bass_utils import run_bass_kernel_spmd

    u = np.asarray(u)
    n_sites = u.shape[1]
    if n_sites not in _BUILD_CACHE:
        _BUILD_CACHE[n_sites] = _build(n_sites)
    nc = _BUILD_CACHE[n_sites]

    packed = _pack_inputs(u, w_ih, w_hh, b_ih, b_hh, head_w, head_b)
    in_maps = _make_in_maps(packed)
    res = run_bass_kernel_spmd(nc, in_maps, list(range(N_CORES)))
    return _assemble_output(res.results, n_sites)


def _assemble_output(results, n_sites):
    out = np.empty((BATCH, n_sites), np.int32)
    for c in range(N_CORES):
        # Device stores s = sign(logit - uhat) in {-1, +1}; bit = (s > 0).
        raw = results[c]["bits"].astype(np.float32)
        out[c * B_LOCAL : (c + 1) * B_LOCAL] = (raw.T > 0).astype(np.int32)
    return out


# revision 19
# speedup vs baseline: 1.0549x; 1.0549x over previous
"""Trainium2 Bass kernel for autoregressive GRU sampling.

Problem: B=16384 samples, 1024 sequential sites, hidden=64, PyTorch GRU-cell
math with gates [r,z,n], Bernoulli sampling via pre-drawn uniforms.

Strategy (v4):
  - Pure data parallel over 8 cores (2048 samples/core); 4 phase-shifted
    streams of 512 columns per core.
  - All matmuls run as bf16 hi/lo pair-split passes (1 cycle/col each,
    vs 8 cycles/col for fp32): X*W ~= Xhi*Whi + Xlo*Whi + Xhi*Wlo, with
    W = Whi + Wlo split exactly on the host and X = h' split on device
    (hi = bf16(h'), lo = bf16(h' - hi)). Error ~2^-18, checked empirically
    against the 2e-2 relative-error budget on the fixed problem seed.
  - Moving tiles per stream (bf16): M1 = [hhi(0-63), s(64), one(65),
    uhat_hi(66)], M2 = [hlo(0-63), 0, 0, uhat_mid(66), uhat_lo(67)].
    Gate passes read rows 0-65 (M2's zero rows avoid double-counting the
    s/one contributions); the head pass reads the uhat rows too, with
    stationary -1 entries so psum = head.h' - uhat (uhat as an exact
    bf16 triple). One ACT Sign then yields s' = sign(logit - uhat).
  - The s in {-1,+1} encoding makes bit*w_ih = s*(w/2) + w/2 ride the
    stationary rows exactly (w/2 is lossless); biases ride row 65.
  - fp32 state h' persists only for the GRU combine (d = h - n).
"""

import numpy as np
from contextlib import ExitStack

HIDDEN = 64
N_SITES = 1024
BATCH = 16384
N_CORES = 8
B_LOCAL = BATCH // N_CORES  # 2048
G = 4                       # streams per core
W = B_LOCAL // G            # 512 columns per stream
NSTAGE = 13
PHASE = (0, 6, 7, 12)       # per-stream stage offsets

_BUILD_CACHE = {}


def _build(n_sites: int, compile: bool = True):
    import concourse.bacc as bacc
    import concourse.tile as tile
    from concourse import mybir

    f32 = mybir.dt.float32
    bf16 = mybir.dt.bfloat16
    AF = mybir.ActivationFunctionType
    OP = mybir.AluOpType

    nc = bacc.Bacc()
    u1_d = nc.dram_tensor("u1", [n_sites, B_LOCAL], bf16, kind="ExternalInput")
    u2_d = nc.dram_tensor("u2", [n_sites, 2, B_LOCAL], bf16, kind="ExternalInput")
    wrzh_d = nc.dram_tensor("wrzh", [66, 128], bf16, kind="ExternalInput")
    wrzl_d = nc.dram_tensor("wrzl", [66, 128], bf16, kind="ExternalInput")
    wnh_d = nc.dram_tensor("wnh", [66, 128], bf16, kind="ExternalInput")
    wnl_d = nc.dram_tensor("wnl", [66, 128], bf16, kind="ExternalInput")
    h1h_d = nc.dram_tensor("h1h", [67, 1], bf16, kind="ExternalInput")
    h1l_d = nc.dram_tensor("h1l", [67, 1], bf16, kind="ExternalInput")
    h2_d = nc.dram_tensor("h2", [68, 1], bf16, kind="ExternalInput")
    bits_d = nc.dram_tensor("bits", [n_sites, B_LOCAL], bf16, kind="ExternalOutput")

    with ExitStack() as ctx:
        tc = ctx.enter_context(tile.TileContext(nc))
        const = ctx.enter_context(tc.tile_pool(name="const", bufs=1))
        work = ctx.enter_context(tc.tile_pool(name="work", bufs=2))
        ps_rz = ctx.enter_context(tc.tile_pool(name="psrz", bufs=1, space="PSUM"))
        ps_n = ctx.enter_context(tc.tile_pool(name="psn", bufs=1, space="PSUM"))
        ps_hd = ctx.enter_context(tc.tile_pool(name="pshd", bufs=1, space="PSUM"))

        # Weights bounce through DVE copies to keep consumer wait-sets small.
        names = [
            ("wrzh", [66, 128]), ("wrzl", [66, 128]),
            ("wnh", [66, 128]), ("wnl", [66, 128]),
            ("h1h", [67, 1]), ("h1l", [67, 1]), ("h2", [68, 1]),
        ]
        drams = dict(wrzh=wrzh_d, wrzl=wrzl_d, wnh=wnh_d, wnl=wnl_d,
                     h1h=h1h_d, h1l=h1l_d, h2=h2_d)
        wt = {}
        for nm, shp in names:
            raw = const.tile(shp, bf16, name=nm + "_raw")
            dst = const.tile(shp, bf16, name=nm)
            nc.sync.dma_start(raw[:], drams[nm][:])
            nc.vector.tensor_copy(dst[:], raw[:])
            wt[nm] = dst

        # fp32 h' state (pure h rows), ping-pong per stream.
        hstate = [
            [const.tile([64, W], f32, name=f"hs{g}_{p}") for p in range(2)]
            for g in range(G)
        ]
        for g in range(G):
            for p in range(2):
                nc.vector.memzero(hstate[g][p][:])

        # M1/M2 as explicit stable double-buffers (slot t -> buffer t%2):
        # stable tensors let Tile see every cross-site WAR/RAW conflict
        # (a rotating pool ring would be blind to readers emitted after the
        # next allocation, which is a real hardware race).
        m1b = [
            [const.tile([67, W], bf16, name=f"m1_{g}_{p}") for p in range(2)]
            for g in range(G)
        ]
        m2b = [
            [const.tile([68, W], bf16, name=f"m2_{g}_{p}") for p in range(2)]
            for g in range(G)
        ]

        def m1s(g, t):
            return m1b[g][t % 2]

        def m2s(g, t):
            return m2b[g][t % 2]

        def uhat_dma(g, t):
            # u2 first so the head M1-pass's SP wait (on u1) also covers u2.
            nc.sync.dma_start(
                m2s(g, t)[66:68, :], u2_d[t : t + 1, :, g * W : (g + 1) * W]
            )
            nc.sync.dma_start(
                m1s(g, t)[66:67, :], u1_d[t : t + 1, g * W : (g + 1) * W]
            )

        # Prologue. Buffer 1 doubles as the initial state (slot -1): h=0,
        # s=-1 (bit 0), one=1. Buffer 0's s-row is overwritten by sign(0)
        # before gates(1) read it, so 1.0 there is fine (32-aligned memsets).
        for g in range(G):
            m1a, m2a = m1b[g][1], m2b[g][1]
            nc.vector.memzero(m1a[0:64, :])
            nc.vector.memset(m1a[64:66, :], 1.0)
            nc.vector.memset(m1a[64:65, :], -1.0)
            nc.vector.memzero(m2a[0:66, :])
            nc.vector.memset(m1b[g][0][64:66, :], 1.0)
            nc.vector.memzero(m2b[g][0][64:66, :])
            uhat_dma(g, 0)
            uhat_dma(g, 1)

        # Stream g's logit-uhat row: tile hd[g//2], row 32*(g%2).
        hd = [ps_hd.tile([64, W], f32, name=f"hd{i}") for i in range(2)]
        tiles = [{} for _ in range(G)]

        def emit(g, t, stage):
            tl = tiles[g]
            if stage == 0:
                m1p, m2p = m1s(g, t - 1), m2s(g, t - 1)
                # M2 pass first: its DVE RAW (hlo) merges with the psum-WAR
                # DVE wait, keeping every matmul within 2 sync waits.
                pn = ps_n.tile([128, W], f32, tag=f"n{g}", name=f"pn{g}")
                tl["pn"] = pn
                nc.tensor.matmul(pn[:], wt["wnh"][:], m2p[0:66, :], start=True, stop=False)
                nc.tensor.matmul(pn[:], wt["wnh"][:], m1p[0:66, :], start=False, stop=False)
                nc.tensor.matmul(pn[:], wt["wnl"][:], m1p[0:66, :], start=False, stop=True)
                prz = ps_rz.tile([128, W], f32, tag=f"rz{g % 2}", name=f"prz{g}")
                tl["prz"] = prz
                nc.tensor.matmul(prz[:], wt["wrzh"][:], m2p[0:66, :], start=True, stop=False)
                nc.tensor.matmul(prz[:], wt["wrzh"][:], m1p[0:66, :], start=False, stop=False)
                nc.tensor.matmul(prz[:], wt["wrzl"][:], m1p[0:66, :], start=False, stop=True)
            elif stage == 1:
                tl["sg"] = work.tile([128, W], f32, tag=f"sg{g}", name=f"sg{g}")
                nc.scalar.activation(tl["sg"][:], tl["prz"][:], AF.Sigmoid)
            elif stage == 2:
                tl["rg"] = work.tile([64, W], f32, tag=f"rg{g}", name=f"rg{g}")
                nc.vector.scalar_tensor_tensor(
                    tl["rg"][:], tl["pn"][0:64, :], 0.0, tl["sg"][64:128, :],
                    OP.add, OP.mult,
                )
            elif stage == 3:
                tl["np"] = work.tile([64, W], f32, tag=f"np{g}", name=f"np{g}")
                nc.vector.tensor_add(tl["np"][:], tl["pn"][64:128, :], tl["rg"][:])
            elif stage == 4:
                tl["nt"] = work.tile([64, W], f32, tag=f"nt{g}", name=f"nt{g}")
                nc.scalar.activation(tl["nt"][:], tl["np"][:], AF.Tanh)
            elif stage == 5:
                tl["d"] = work.tile([64, W], f32, tag=f"d{g}", name=f"d{g}")
                nc.vector.tensor_sub(tl["d"][:], hstate[g][t % 2][:], tl["nt"][:])
            elif stage == 6:
                tl["zd"] = work.tile([64, W], f32, tag=f"zd{g}", name=f"zd{g}")
                nc.gpsimd.tensor_mul(tl["zd"][:], tl["sg"][0:64, :], tl["d"][:])
            elif stage == 7:
                nc.vector.tensor_add(
                    hstate[g][(t + 1) % 2][:], tl["nt"][:], tl["zd"][:]
                )
            elif stage == 8:
                # hhi = bf16(h') into this site's M1 (allocated at stage 12
                # of site t-2, or in the prologue).
                nc.scalar.activation(
                    m1s(g, t)[0:64, :], hstate[g][(t + 1) % 2][:], AF.Copy
                )
            elif stage == 9:
                nc.vector.tensor_sub(
                    m2s(g, t)[0:64, :], hstate[g][(t + 1) % 2][:], m1s(g, t)[0:64, :]
                )
            elif stage == 10:
                row = 32 * (g % 2)
                o = hd[g // 2][row : row + 1, :]
                nc.tensor.matmul(o, wt["h1h"][:], m1s(g, t)[0:67, :], start=True, stop=False)
                nc.tensor.matmul(o, wt["h2"][:], m2s(g, t)[0:68, :], start=False, stop=False)
                nc.tensor.matmul(o, wt["h1l"][:], m1s(g, t)[0:67, :], start=False, stop=True)
            elif stage == 11:
                row = 32 * (g % 2)
                nc.scalar.activation(
                    m1s(g, t)[64:65, :], hd[g // 2][row : row + 1, :], AF.Sign
                )
            elif stage == 12:
                nc.sync.dma_start(
                    bits_d[t : t + 1, g * W : (g + 1) * W], m1s(g, t)[64:65, :]
                )
                if t + 2 < n_sites:
                    uhat_dma(g, t + 2)

        total = n_sites * NSTAGE
        for tick in range(total + PHASE[-1] + 1):
            for g in range(G):
                k = tick - PHASE[g]
                if 0 <= k < total:
                    t, stage = divmod(k, NSTAGE)
                    emit(g, t, stage)

    if compile:
        nc.compile()
    return nc


def _bf16_split(a32):
    import ml_dtypes

    hi = a32.astype(ml_dtypes.bfloat16)
    lo = (a32 - hi.astype(np.float32)).astype(ml_dtypes.bfloat16)
    return hi, lo


def _pack_inputs(u, w_ih, w_hh, b_ih, b_hh, head_w, head_b):
    import ml_dtypes

    H = HIDDEN
    w_ih = np.asarray(w_ih, np.float64)
    w_hh = np.asarray(w_hh, np.float32)
    b_ih = np.asarray(b_ih, np.float64)
    b_hh = np.asarray(b_hh, np.float64)
    head_w = np.asarray(head_w, np.float32)
    head_b = np.asarray(head_b, np.float64)

    r_sl = slice(0, H)
    z_sl = slice(H, 2 * H)
    n_sl = slice(2 * H, 3 * H)

    # Row 64 multiplies s=2*bit-1, row 65 multiplies const 1:
    #   bit*w = s*(w/2) + w/2, so row64 = w/2 and +w/2 joins the bias.
    wrz = np.zeros((66, 128), np.float32)
    wrz[0:64, 0:64] = w_hh[z_sl].T
    wrz[64, 0:64] = (w_ih[z_sl, 0] / 2).astype(np.float32)
    wrz[65, 0:64] = (b_ih[z_sl] + b_hh[z_sl] + w_ih[z_sl, 0] / 2).astype(np.float32)
    wrz[0:64, 64:128] = w_hh[r_sl].T
    wrz[64, 64:128] = (w_ih[r_sl, 0] / 2).astype(np.float32)
    wrz[65, 64:128] = (b_ih[r_sl] + b_hh[r_sl] + w_ih[r_sl, 0] / 2).astype(np.float32)

    wn = np.zeros((66, 128), np.float32)
    wn[0:64, 0:64] = w_hh[n_sl].T
    wn[65, 0:64] = b_hh[n_sl].astype(np.float32)
    wn[64, 64:128] = (w_ih[n_sl, 0] / 2).astype(np.float32)
    wn[65, 64:128] = (b_ih[n_sl] + w_ih[n_sl, 0] / 2).astype(np.float32)

    wrzh, wrzl = _bf16_split(wrz)
    wnh, wnl = _bf16_split(wn)

    headh, headl = _bf16_split(head_w[0, :].astype(np.float32))
    h1h = np.zeros((67, 1), ml_dtypes.bfloat16)
    h1h[0:64, 0] = headh
    h1h[66, 0] = ml_dtypes.bfloat16(-1.0)
    h1l = np.zeros((67, 1), ml_dtypes.bfloat16)
    h1l[0:64, 0] = headl
    h2 = np.zeros((68, 1), ml_dtypes.bfloat16)
    h2[0:64, 0] = headh
    h2[66, 0] = ml_dtypes.bfloat16(-1.0)
    h2[67, 0] = ml_dtypes.bfloat16(-1.0)

    # uhat = logit(u) - head_b as an exact bf16 triple.
    u64 = np.asarray(u, np.float64)
    L = (np.log(u64) - np.log1p(-u64) - float(head_b[0])).astype(np.float32)
    uhi = L.astype(ml_dtypes.bfloat16)
    rem = L - uhi.astype(np.float32)
    umid = rem.astype(ml_dtypes.bfloat16)
    ulo = (rem - umid.astype(np.float32)).astype(ml_dtypes.bfloat16)

    u1s, u2s = [], []
    for c in range(N_CORES):
        sl = slice(c * B_LOCAL, (c + 1) * B_LOCAL)
        u1s.append(np.ascontiguousarray(uhi[sl].T))
        u2s.append(
            np.ascontiguousarray(
                np.stack([umid[sl].T, ulo[sl].T], axis=1)  # [n_sites, 2, BL]
            )
        )
    return wrzh, wrzl, wnh, wnl, h1h, h1l, h2, u1s, u2s


def _make_in_maps(packed):
    wrzh, wrzl, wnh, wnl, h1h, h1l, h2, u1s, u2s = packed
    return [
        {
            "u1": u1s[c], "u2": u2s[c], "wrzh": wrzh, "wrzl": wrzl,
            "wnh": wnh, "wnl": wnl, "h1h": h1h, "h1l": h1l, "h2": h2,
        }
        for c in range(N_CORES)
    ]


def kernel(u, w_ih, w_hh, b_ih, b_hh, head_w, head_b):
    from concourse.bass_utils import run_bass_kernel_spmd

    u = np.asarray(u)
    n_sites = u.shape[1]
    if n_sites not in _BUILD_CACHE:
        _BUILD_CACHE[n_sites] = _build(n_sites)
    nc = _BUILD_CACHE[n_sites]

    packed = _pack_inputs(u, w_ih, w_hh, b_ih, b_hh, head_w, head_b)
    in_maps = _make_in_maps(packed)
    res = run_bass_kernel_spmd(nc, in_maps, list(range(N_CORES)))
    return _assemble_output(res.results, n_sites)


def _assemble_output(results, n_sites):
    out = np.empty((BATCH, n_sites), np.int32)
    for c in range(N_CORES):
        # Device stores s = sign(logit - uhat) in {-1, +1}; bit = (s > 0).
        raw = results[c]["bits"].astype(np.float32)
        out[c * B_LOCAL : (c + 1) * B_LOCAL] = (raw.T > 0).astype(np.int32)
    return out
